# revision 28
# baseline (speedup 1.0000x reference)
"""GCN denoise net (2-layer GCNConv + time MLP) on 8 Trainium2 NeuronCores.

Strategy (v2 — descriptor-generation-bound design):
  - Aggregate-then-transform: out = (A_hat @ x) @ W.T + b, exploiting linearity.
  - Nodes permuted into 50176 "token" slots (392 windows of 128) with a
    degree-balanced serpentine assignment, so every (window, source-half)
    edge group fits M=7 blocks of 128 -> minimal gather padding.
  - Targets sharded: core c owns windows [c*49, (c+1)*49) = tokens
    [c*6272, (c+1)*6272).
  - Edge aggregation per 128-target window via PE one-hot matmuls:
    cur[f,t] += zg[e,f].T-style accumulate with stiles stc[e,t]=norm.
  - Source features fetched by gpsimd dma_gather (1024-idx calls, the HW
    max) from a row-major token table in HBM; descriptors for self-loops
    are avoided entirely: the diagonal (self) contribution is one dense
    matmul per window against the contiguously-loaded own-shard tile.
  - Stiles are host-transposed to [128e, NBT*128t] so they stream as
    2KB-contiguous descriptors instead of 256B ones.
  - temb (time-MLP table, b1 folded) preloaded once in wrapped layout.
  - h1 exchanged between layers with an on-device AllGather; each core's
    own h1 windows stay resident in SBUF for the layer-2 self matmuls.
"""

import sys
import numpy as np

sys.path.insert(0, "/opt/trn_rl_repo")

# ---------------- problem constants (hardcoded per contract) ----------------
N = 50000
E = 640000
D = 128
CORES = 8
NW = 392                      # 128-token windows total
NTOK = NW * 128               # 50176 padded tokens
WPC = NW // CORES             # 49 windows per core
SHARD = WPC * 128             # 6272 tokens per core
HALF = NTOK // 2              # 25088
G = 8                         # gather chunk size in blocks (1024 idxs = HW max)
TMAX = 1000


def _prep(z, edge_index, t):
    """Host preprocessing: degrees, norms, balanced permutation, edge
    partitioning, transposed stiles."""
    row = np.asarray(edge_index[0], dtype=np.int64)
    col = np.asarray(edge_index[1], dtype=np.int64)

    deg = np.bincount(col, minlength=N).astype(np.float64) + 1.0
    dinv = (1.0 / np.sqrt(deg)).astype(np.float32)

    # --- balanced node->token permutation (serpentine over in-degree) ---
    indeg = np.bincount(col, minlength=N)
    order = np.argsort(-indeg, kind="stable")
    i = np.arange(N)
    r = i // NW
    j = i % NW
    win = np.where(r % 2 == 0, j, NW - 1 - j)
    token = win * 128 + r
    tok = np.empty(N, dtype=np.int64)
    tok[order] = token
    inv_tok = np.full(NTOK, -1, dtype=np.int64)
    inv_tok[tok] = np.arange(N)

    tr = tok[row]
    tc = tok[col]
    norm = (dinv[row] * dinv[col]).astype(np.float32)

    core_of = tc // SHARD
    per_core_raw = []
    M = 1
    for k in range(CORES):
        sel = core_of == k
        trk = tr[sel]
        tck = tc[sel]
        nk = norm[sel]
        wloc = (tck - k * SHARD) >> 7
        coff = tck & 127
        half = (trk >= HALF).astype(np.int64)
        sidx = trk - half * HALF
        key = half * WPC + wloc
        o2 = np.argsort(key, kind="stable")
        sidx, coff, nk, key = sidx[o2], coff[o2], nk[o2], key[o2]
        cnt = np.bincount(key, minlength=2 * WPC)
        M = max(M, int(np.ceil(cnt.max() / 128)))
        per_core_raw.append((sidx, coff, nk, cnt))

    NBH = WPC * M             # blocks per half
    NBT = 2 * NBH             # blocks per layer schedule

    # --- layer-2 two-piece source split (pipelined AllGather), B=24 ---
    PB = [0, 24, WPC]
    src_core = tr // SHARD
    wl_src = (tr % SHARD) >> 7
    piece_of = (wl_src >= PB[1]).astype(np.int64)
    pw = np.array([PB[1], WPC - PB[1]])
    idx2_all = (src_core * (pw[piece_of] * 128)
                + (wl_src - np.array(PB)[piece_of]) * 128 + (tr & 127))

    per_core_raw2 = []
    M2 = 1
    for k in range(CORES):
        sel = core_of == k
        wloc = (tc[sel] - k * SHARD) >> 7
        key2 = piece_of[sel] * WPC + wloc
        o3 = np.argsort(key2, kind="stable")
        i2 = idx2_all[sel][o3]
        nk2 = norm[sel][o3]
        cf2 = (tc[sel] & 127)[o3]
        cnt2 = np.bincount(key2[o3], minlength=2 * WPC)
        M2 = max(M2, int(np.ceil(cnt2.max() / 128)))
        per_core_raw2.append((i2, cf2, nk2, cnt2))
    NBP = WPC * M2
    NBT2 = 2 * NBP

    core_inputs = []
    for k in range(CORES):
        sidx, coff, nk, cnt = per_core_raw[k]
        s_arr = np.zeros(NBT * 128, dtype=np.int16)
        c_arr = np.zeros(NBT * 128, dtype=np.int64)
        n_arr = np.zeros(NBT * 128, dtype=np.float32)
        starts = np.concatenate([[0], np.cumsum(cnt)])
        for g in range(2 * WPC):
            m = int(cnt[g])
            if m == 0:
                continue
            src = slice(starts[g], starts[g] + m)
            dst = slice(g * M * 128, g * M * 128 + m)
            s_arr[dst] = sidx[src].astype(np.int16)
            c_arr[dst] = coff[src]
            n_arr[dst] = nk[src]

        # transposed stiles: stT[e, b*128 + t] = norm of slot (b,e) -> t
        st_flat = np.zeros((NBT * 128, 128), dtype=np.float16)
        st_flat[np.arange(NBT * 128), c_arr] = n_arr.astype(np.float16)
        stT = np.ascontiguousarray(
            st_flat.reshape(NBT, 128, 128).transpose(1, 0, 2).reshape(128, NBT * 128))

        # wrapped gather-index layout: unwrapped[i] = tile[i % 16, i // 16],
        # replicated across the 8 groups of 16 partitions
        wr = np.ascontiguousarray(s_arr.reshape(NBT * 8, 16).T)  # [16, NBT*8]
        idx_t = np.ascontiguousarray(np.tile(wr, (8, 1)))        # [128, NBT*8]

        # layer-2 piece-based schedule
        i2, cf2, nk2, cnt2 = per_core_raw2[k]
        s2_arr = np.zeros(NBT2 * 128, dtype=np.int16)
        c2_arr = np.zeros(NBT2 * 128, dtype=np.int64)
        n2_arr = np.zeros(NBT2 * 128, dtype=np.float32)
        starts2 = np.concatenate([[0], np.cumsum(cnt2)])
        for g in range(2 * WPC):
            m2_ = int(cnt2[g])
            if m2_ == 0:
                continue
            src2 = slice(starts2[g], starts2[g] + m2_)
            dst2 = slice(g * M2 * 128, g * M2 * 128 + m2_)
            s2_arr[dst2] = i2[src2].astype(np.int16)
            c2_arr[dst2] = cf2[src2]
            n2_arr[dst2] = nk2[src2]
        st2_flat = np.zeros((NBT2 * 128, 128), dtype=np.float16)
        st2_flat[np.arange(NBT2 * 128), c2_arr] = n2_arr.astype(np.float16)
        st2T = np.ascontiguousarray(
            st2_flat.reshape(NBT2, 128, 128).transpose(1, 0, 2).reshape(128, NBT2 * 128))
        wr2 = np.ascontiguousarray(s2_arr.reshape(NBT2 * 8, 16).T)
        idx2_t = np.ascontiguousarray(np.tile(wr2, (8, 1)))      # [128, NBT2*8]

        # self-loop diagonal [r, w*128+t]: dinv^2 of own token (w*128+r)
        own = inv_tok[k * SHARD:(k + 1) * SHARD]                 # [6272]
        dg = np.zeros(SHARD, dtype=np.float16)
        ok = own >= 0
        dg[ok] = (dinv[own[ok]] ** 2).astype(np.float16)
        diagT = np.zeros((128, SHARD), dtype=np.float16)
        p = np.arange(SHARD)
        diagT[p & 127, (p >> 7) * 128 + (p & 127)] = dg

        # own z rows, wrapped: zself[p, w*128+f] = z[token w*128+p]
        zself = np.zeros((128, SHARD), dtype=np.float16)
        zs = np.zeros((SHARD, D), dtype=np.float16)
        zs[ok] = z[own[ok]].astype(np.float16)
        zself[:, :] = zs.reshape(WPC, 128, D).transpose(1, 0, 2).reshape(128, SHARD)

        core_inputs.append({
            "stilesT": stT,
            "idx16": idx_t,
            "stiles2T": st2T,
            "idx2": idx2_t,
            "diagT": diagT,
            "zself": zself,
            "_own": own,
        })
    return (M, M2), tok, inv_tok, dinv, core_inputs


_BUILD_CACHE = {}
LAST_RESULT = None


def _build(Ms):
    """Build the SPMD Bass program (one NEFF shared by all 8 cores)."""
    M, M2 = Ms
    import os as _os
    no_cc = bool(int(_os.environ.get("KERNEL_NO_CC", "0")))
    key = (M, M2, no_cc)
    if key in _BUILD_CACHE:
        return _BUILD_CACHE[key]

    import concourse.bass as bass
    import concourse.mybir as mybir
    import concourse.tile as tile
    from concourse.bass import ts

    f16 = mybir.dt.float16
    f32 = mybir.dt.float32
    i16 = mybir.dt.int16
    AF = mybir.ActivationFunctionType
    OP = mybir.AluOpType

    NBH = WPC * M
    NBT = 2 * NBH
    PB = [0, 24, WPC]
    NBP = WPC * M2
    NBT2 = 2 * NBP

    from concourse import bacc
    nc = bacc.Bacc(num_devices=CORES, num_swdge_queues=4)

    ztab_d = nc.dram_tensor("ztab", [NTOK, D], f16, kind="ExternalInput")
    idx_d = nc.dram_tensor("idx16", [128, NBT * 8], i16, kind="ExternalInput")
    stiles_d = nc.dram_tensor("stilesT", [128, NBT * 128], f16, kind="ExternalInput")
    temb_d = nc.dram_tensor("temb", [128, SHARD], f16, kind="ExternalInput")
    diag_d = nc.dram_tensor("diagT", [128, SHARD], f16, kind="ExternalInput")
    zself_d = nc.dram_tensor("zself", [128, SHARD], f16, kind="ExternalInput")
    w1t_d = nc.dram_tensor("w1t", [D, D], f16, kind="ExternalInput")
    w2t_d = nc.dram_tensor("w2t", [D, D], f16, kind="ExternalInput")
    bias2_d = nc.dram_tensor("bias2", [1, D], f16, kind="ExternalInput")
    ones_d = nc.dram_tensor("ones", [1, D], f16, kind="ExternalInput")

    idx2_d = nc.dram_tensor("idx2", [128, NBT2 * 8], i16, kind="ExternalInput")
    stiles2_d = nc.dram_tensor("stiles2T", [128, NBT2 * 128], f16,
                               kind="ExternalInput")

    out_d = nc.dram_tensor("outshard", [SHARD, D], f32, kind="ExternalOutput")
    h1sp_d = [nc.dram_tensor(f"h1sp{q}", [(PB[q + 1] - PB[q]) * 128, D], f16)
              for q in range(2)]
    h1pf_d = [nc.dram_tensor(f"h1pf{q}", [CORES * (PB[q + 1] - PB[q]) * 128, D],
                             f16, addr_space="Shared") for q in range(2)]

    with tile.TileContext(nc) as tc:
        with (
            tc.tile_pool(name="const", bufs=1) as constp,
            tc.tile_pool(name="work", bufs=2) as workp,
            tc.tile_pool(name="psum", bufs=2, space="PSUM") as psump,
        ):
            def load_const(dram, shape, dtype, eng=None):
                t_ = constp.tile(shape, dtype, name=dram.name + "_t")
                (eng or nc.sync).dma_start(out=t_[:], in_=dram[:, :])
                return t_

            idx_t = load_const(idx_d, [128, NBT * 8], i16)
            idx2_t = load_const(idx2_d, [128, NBT2 * 8], i16)
            w1t_t = load_const(w1t_d, [D, D], f16)
            w2t_t = load_const(w2t_d, [D, D], f16)
            bias2_t = load_const(bias2_d, [1, D], f16)
            ones_t = load_const(ones_d, [1, D], f16)
            temb_t = load_const(temb_d, [128, SHARD], f16)
            diag_t = load_const(diag_d, [128, SHARD], f16)
            zself_t = load_const(zself_d, [128, SHARD], f16)

            aggbuf = constp.tile([128, SHARD], f32, name="aggbuf")
            h1stage = constp.tile([128, SHARD], f16, name="h1stage")

            def transform(L, w, aggT):
                # aggT: [f, t] f16; hp = aggT.T @ W.T -> [t, f']
                hp = psump.tile([128, D], f32, tag="hp", name="hp")
                wm = w1t_t if L == 0 else w2t_t
                nc.tensor.matmul(hp[:], lhsT=aggT[:], rhs=wm[:],
                                 start=True, stop=(L == 0))
                if L == 0:
                    # x = conv + temb ; h1 = elu(x) = relu(x)+exp(min(x,0))-1
                    x = workp.tile([128, D], f32, tag="x", name="x")
                    nc.vector.tensor_tensor(out=x[:], in0=hp[:],
                                            in1=temb_t[:, ts(w, D)], op=OP.add)
                    mn = workp.tile([128, D], f32, tag="mn", name="mn")
                    nc.vector.tensor_scalar(out=mn[:], in0=x[:],
                                            scalar1=0.0, scalar2=-60.0,
                                            op0=OP.min, op1=OP.max)
                    ex = workp.tile([128, D], f16, tag="ex", name="ex")
                    nc.scalar.activation(ex[:], mn[:], AF.Exp)
                    rl = workp.tile([128, D], f16, tag="rl", name="rl")
                    nc.scalar.activation(rl[:], x[:], AF.Relu)
                    e1 = workp.tile([128, D], f16, tag="e1", name="e1")
                    nc.vector.tensor_scalar(out=e1[:], in0=ex[:],
                                            scalar1=1.0, scalar2=None,
                                            op0=OP.subtract)
                    nc.vector.tensor_tensor(out=h1stage[:, ts(w, D)],
                                            in0=e1[:], in1=rl[:], op=OP.add)
                    p_ = 0 if w < PB[1] else 1
                    lw = w - PB[p_]
                    nc.sync.dma_start(
                        out=h1sp_d[p_][lw * 128:(lw + 1) * 128, :],
                        in_=h1stage[:, ts(w, D)])
                    if w == PB[p_ + 1] - 1:
                        if no_cc:
                            rows_ = (PB[p_ + 1] - PB[p_]) * 128
                            nc.sync.dma_start(out=h1pf_d[p_][0:rows_, :],
                                              in_=h1sp_d[p_][:, :])
                        else:
                            nc.gpsimd.collective_compute(
                                "AllGather",
                                mybir.AluOpType.bypass,
                                replica_groups=[list(range(CORES))],
                                ins=[h1sp_d[p_].ap().opt()],
                                outs=[h1pf_d[p_].ap().opt()],
                            )
                else:
                    nc.tensor.matmul(hp[:], lhsT=ones_t[:1, :],
                                     rhs=bias2_t[:1, :],
                                     start=False, stop=True)
                    ot = workp.tile([128, D], f32, tag="ot", name="ot")
                    nc.scalar.copy(out=ot[:], in_=hp[:])
                    nc.sync.dma_start(out=out_d[w * 128:(w + 1) * 128, :],
                                      in_=ot[:])

            gci = 0
            for L in range(2):
                selfsrc = zself_t if L == 0 else h1stage
                Mc = M if L == 0 else M2
                NBc = WPC * Mc
                idxc = idx_t if L == 0 else idx2_t
                stdram = stiles_d if L == 0 else stiles2_d
                for h in range(2):
                    if L == 0:
                        src_ap = ztab_d[:, :] if h == 0 else ztab_d[HALF:, :]
                    else:
                        src_ap = h1pf_d[h][:, :]
                    cur = None
                    stc = None
                    for ci_, c0 in enumerate(range(0, NBc, G)):
                        nbc = min(G, NBc - c0)
                        gb = h * NBc + c0
                        zg = workp.tile([128, nbc * D], f16, tag="zg",
                                        bufs=20, name="zg")
                        nc.gpsimd.dma_gather(
                            out_ap=zg[:].rearrange("p (b e) -> p b e", e=D),
                            in_ap=src_ap,
                            idxs_ap=idxc[:, gb * 8:(gb + nbc) * 8],
                            num_idxs=nbc * 128,
                            num_idxs_reg=nbc * 128,
                            elem_size=D,
                            queue_num=gci % 4,
                        )
                        gci += 1
                        if ci_ % 2 == 0:
                            nst = min(2 * G, NBc - c0)
                            stc = workp.tile([128, nst * D], f16, tag="stc",
                                             bufs=8, name="stc")
                            eng = nc.sync if (ci_ // 2) % 2 == 0 else nc.scalar
                            eng.dma_start(
                                out=stc[:],
                                in_=stdram[:, gb * 128:(gb + nst) * 128],
                            )
                            stc_base = c0
                        for jj in range(nbc):
                            blk = c0 + jj
                            sj = blk - stc_base
                            w = blk // Mc
                            jm = blk % Mc
                            if jm == 0:
                                cur = psump.tile([128, D], f32, tag="agg",
                                                 bufs=6, name="agg")
                                if h == 0:
                                    nc.tensor.matmul(
                                        cur[:], lhsT=selfsrc[:, ts(w, D)],
                                        rhs=diag_t[:, ts(w, D)],
                                        start=True, stop=False)
                            nc.tensor.matmul(
                                cur[:], lhsT=zg[:, ts(jj, D)],
                                rhs=stc[:, ts(sj, D)],
                                start=(jm == 0 and h == 1),
                                stop=(jm == Mc - 1),
                            )
                            if jm == Mc - 1:
                                if h == 0:
                                    nc.scalar.copy(out=aggbuf[:, ts(w, D)],
                                                   in_=cur[:])
                                else:
                                    aggT = workp.tile([128, D], f16,
                                                      tag="aggT", bufs=3,
                                                      name="aggT")
                                    nc.vector.tensor_tensor(
                                        out=aggT[:], in0=cur[:],
                                        in1=aggbuf[:, ts(w, D)], op=OP.add)
                                    transform(L, w, aggT)

    nc.finalize()
    _BUILD_CACHE[key] = nc
    return nc


def _temb_table(Wt1, bt1, Wt2, bt2, b1):
    """Exact time-MLP for all possible t values: [TMAX, D] float64 -> f32."""
    try:
        from scipy.special import erf
    except ImportError:
        import math
        erf = np.vectorize(math.erf)
    tv = np.arange(TMAX, dtype=np.float64)[:, None]          # [TMAX, 1]
    pre = tv @ np.asarray(Wt1, np.float64).T + np.asarray(bt1, np.float64)
    g = 0.5 * pre * (1.0 + erf(pre / np.sqrt(2.0)))
    emb = g @ np.asarray(Wt2, np.float64).T + np.asarray(bt2, np.float64)
    emb = emb + np.asarray(b1, np.float64)
    return emb.astype(np.float32)


def kernel(z, edge_index, t, Wt1, bt1, Wt2, bt2, W1, b1, W2, b2):
    z = np.asarray(z, dtype=np.float32)
    t = np.asarray(t)
    Ms, tok, inv_tok, dinv, core_inputs = _prep(z, edge_index, t)
    nc = _build(Ms)

    table = _temb_table(Wt1, bt1, Wt2, bt2, b1)

    ztab = np.zeros((NTOK, D), dtype=np.float16)
    ztab[tok] = z.astype(np.float16)

    shared = {
        "ztab": ztab,
        "w1t": np.ascontiguousarray(np.asarray(W1, np.float32).T).astype(np.float16),
        "w2t": np.ascontiguousarray(np.asarray(W2, np.float32).T).astype(np.float16),
        "bias2": np.asarray(b2, np.float32).astype(np.float16).reshape(1, D),
        "ones": np.ones((1, D), dtype=np.float16),
    }

    in_maps = []
    for k in range(CORES):
        ci = core_inputs[k]
        m = dict(shared)
        m["stilesT"] = ci["stilesT"]
        m["idx16"] = ci["idx16"]
        m["stiles2T"] = ci["stiles2T"]
        m["idx2"] = ci["idx2"]
        m["diagT"] = ci["diagT"]
        m["zself"] = ci["zself"]
        own = ci["_own"]
        te = np.zeros((SHARD, D), dtype=np.float16)
        ok = own >= 0
        te[ok] = table[np.asarray(t[own[ok]], np.int64)].astype(np.float16)
        m["temb"] = np.ascontiguousarray(
            te.reshape(WPC, 128, D).transpose(1, 0, 2).reshape(128, SHARD))
        in_maps.append(m)

    from concourse.bass_utils import run_bass_kernel_spmd
    res = run_bass_kernel_spmd(nc, in_maps, core_ids=list(range(CORES)))
    global LAST_RESULT
    LAST_RESULT = res
    out_tok = np.concatenate(
        [res.results[k]["outshard"] for k in range(CORES)], axis=0)
    return out_tok[tok].astype(np.float32)


# revision 29
# speedup vs baseline: 1.0207x; 1.0207x over previous
"""GCN denoise net (2-layer GCNConv + time MLP) on 8 Trainium2 NeuronCores.

Strategy (v2 — descriptor-generation-bound design):
  - Aggregate-then-transform: out = (A_hat @ x) @ W.T + b, exploiting linearity.
  - Nodes permuted into 50176 "token" slots (392 windows of 128) with a
    degree-balanced serpentine assignment, so every (window, source-half)
    edge group fits M=7 blocks of 128 -> minimal gather padding.
  - Targets sharded: core c owns windows [c*49, (c+1)*49) = tokens
    [c*6272, (c+1)*6272).
  - Edge aggregation per 128-target window via PE one-hot matmuls:
    cur[f,t] += zg[e,f].T-style accumulate with stiles stc[e,t]=norm.
  - Source features fetched by gpsimd dma_gather (1024-idx calls, the HW
    max) from a row-major token table in HBM; descriptors for self-loops
    are avoided entirely: the diagonal (self) contribution is one dense
    matmul per window against the contiguously-loaded own-shard tile.
  - Stiles are host-transposed to [128e, NBT*128t] so they stream as
    2KB-contiguous descriptors instead of 256B ones.
  - temb (time-MLP table, b1 folded) preloaded once in wrapped layout.
  - h1 exchanged between layers with an on-device AllGather; each core's
    own h1 windows stay resident in SBUF for the layer-2 self matmuls.
"""

import sys
import numpy as np

sys.path.insert(0, "/opt/trn_rl_repo")

# ---------------- problem constants (hardcoded per contract) ----------------
N = 50000
E = 640000
D = 128
CORES = 8
NW = 392                      # 128-token windows total
NTOK = NW * 128               # 50176 padded tokens
WPC = NW // CORES             # 49 windows per core
SHARD = WPC * 128             # 6272 tokens per core
HALF = NTOK // 2              # 25088
G = 8                         # gather chunk size in blocks (1024 idxs = HW max)
TMAX = 1000


def _prep(z, edge_index, t):
    """Host preprocessing: degrees, norms, balanced permutation, edge
    partitioning, transposed stiles."""
    row = np.asarray(edge_index[0], dtype=np.int64)
    col = np.asarray(edge_index[1], dtype=np.int64)

    deg = np.bincount(col, minlength=N).astype(np.float64) + 1.0
    dinv = (1.0 / np.sqrt(deg)).astype(np.float32)

    # --- balanced node->token permutation (serpentine over in-degree) ---
    indeg = np.bincount(col, minlength=N)
    order = np.argsort(-indeg, kind="stable")
    i = np.arange(N)
    r = i // NW
    j = i % NW
    win = np.where(r % 2 == 0, j, NW - 1 - j)
    token = win * 128 + r
    tok = np.empty(N, dtype=np.int64)
    tok[order] = token
    inv_tok = np.full(NTOK, -1, dtype=np.int64)
    inv_tok[tok] = np.arange(N)

    tr = tok[row]
    tc = tok[col]
    norm = (dinv[row] * dinv[col]).astype(np.float32)

    core_of = tc // SHARD
    per_core_raw = []
    M = 1
    for k in range(CORES):
        sel = core_of == k
        trk = tr[sel]
        tck = tc[sel]
        nk = norm[sel]
        wloc = (tck - k * SHARD) >> 7
        coff = tck & 127
        half = (trk >= HALF).astype(np.int64)
        sidx = trk - half * HALF
        key = half * WPC + wloc
        o2 = np.argsort(key, kind="stable")
        sidx, coff, nk, key = sidx[o2], coff[o2], nk[o2], key[o2]
        cnt = np.bincount(key, minlength=2 * WPC)
        M = max(M, int(np.ceil(cnt.max() / 128)))
        per_core_raw.append((sidx, coff, nk, cnt))

    NBH = WPC * M             # blocks per half
    NBT = 2 * NBH             # blocks per layer schedule

    # --- layer-2 two-piece source split (pipelined AllGather), B=24 ---
    PB = [0, 24, WPC]
    src_core = tr // SHARD
    wl_src = (tr % SHARD) >> 7
    piece_of = (wl_src >= PB[1]).astype(np.int64)
    pw = np.array([PB[1], WPC - PB[1]])
    idx2_all = (src_core * (pw[piece_of] * 128)
                + (wl_src - np.array(PB)[piece_of]) * 128 + (tr & 127))

    per_core_raw2 = []
    M2 = 1
    for k in range(CORES):
        sel = core_of == k
        wloc = (tc[sel] - k * SHARD) >> 7
        key2 = piece_of[sel] * WPC + wloc
        o3 = np.argsort(key2, kind="stable")
        i2 = idx2_all[sel][o3]
        nk2 = norm[sel][o3]
        cf2 = (tc[sel] & 127)[o3]
        cnt2 = np.bincount(key2[o3], minlength=2 * WPC)
        M2 = max(M2, int(np.ceil(cnt2.max() / 128)))
        per_core_raw2.append((i2, cf2, nk2, cnt2))
    NBP = WPC * M2
    NBT2 = 2 * NBP

    core_inputs = []
    for k in range(CORES):
        sidx, coff, nk, cnt = per_core_raw[k]
        s_arr = np.zeros(NBT * 128, dtype=np.int16)
        c_arr = np.zeros(NBT * 128, dtype=np.int64)
        n_arr = np.zeros(NBT * 128, dtype=np.float32)
        starts = np.concatenate([[0], np.cumsum(cnt)])
        for g in range(2 * WPC):
            m = int(cnt[g])
            if m == 0:
                continue
            src = slice(starts[g], starts[g] + m)
            dst = slice(g * M * 128, g * M * 128 + m)
            s_arr[dst] = sidx[src].astype(np.int16)
            c_arr[dst] = coff[src]
            n_arr[dst] = nk[src]

        # transposed stiles: stT[e, b*128 + t] = norm of slot (b,e) -> t
        st_flat = np.zeros((NBT * 128, 128), dtype=np.float16)
        st_flat[np.arange(NBT * 128), c_arr] = n_arr.astype(np.float16)
        stT = np.ascontiguousarray(
            st_flat.reshape(NBT, 128, 128).transpose(1, 0, 2).reshape(128, NBT * 128))

        # wrapped gather-index layout: unwrapped[i] = tile[i % 16, i // 16],
        # replicated across the 8 groups of 16 partitions
        wr = np.ascontiguousarray(s_arr.reshape(NBT * 8, 16).T)  # [16, NBT*8]
        idx_t = np.ascontiguousarray(np.tile(wr, (8, 1)))        # [128, NBT*8]

        # layer-2 piece-based schedule
        i2, cf2, nk2, cnt2 = per_core_raw2[k]
        s2_arr = np.zeros(NBT2 * 128, dtype=np.int16)
        c2_arr = np.zeros(NBT2 * 128, dtype=np.int64)
        n2_arr = np.zeros(NBT2 * 128, dtype=np.float32)
        starts2 = np.concatenate([[0], np.cumsum(cnt2)])
        for g in range(2 * WPC):
            m2_ = int(cnt2[g])
            if m2_ == 0:
                continue
            src2 = slice(starts2[g], starts2[g] + m2_)
            dst2 = slice(g * M2 * 128, g * M2 * 128 + m2_)
            s2_arr[dst2] = i2[src2].astype(np.int16)
            c2_arr[dst2] = cf2[src2]
            n2_arr[dst2] = nk2[src2]
        st2_flat = np.zeros((NBT2 * 128, 128), dtype=np.float16)
        st2_flat[np.arange(NBT2 * 128), c2_arr] = n2_arr.astype(np.float16)
        st2T = np.ascontiguousarray(
            st2_flat.reshape(NBT2, 128, 128).transpose(1, 0, 2).reshape(128, NBT2 * 128))
        wr2 = np.ascontiguousarray(s2_arr.reshape(NBT2 * 8, 16).T)
        idx2_t = np.ascontiguousarray(np.tile(wr2, (8, 1)))      # [128, NBT2*8]

        # self-loop diagonal [r, w*128+t]: dinv^2 of own token (w*128+r)
        own = inv_tok[k * SHARD:(k + 1) * SHARD]                 # [6272]
        dg = np.zeros(SHARD, dtype=np.float16)
        ok = own >= 0
        dg[ok] = (dinv[own[ok]] ** 2).astype(np.float16)
        diagT = np.zeros((128, SHARD), dtype=np.float16)
        p = np.arange(SHARD)
        diagT[p & 127, (p >> 7) * 128 + (p & 127)] = dg

        # own z rows, wrapped: zself[p, w*128+f] = z[token w*128+p]
        zself = np.zeros((128, SHARD), dtype=np.float16)
        zs = np.zeros((SHARD, D), dtype=np.float16)
        zs[ok] = z[own[ok]].astype(np.float16)
        zself[:, :] = zs.reshape(WPC, 128, D).transpose(1, 0, 2).reshape(128, SHARD)

        core_inputs.append({
            "stilesT": stT,
            "idx16": idx_t,
            "stiles2T": st2T,
            "idx2": idx2_t,
            "diagT": diagT,
            "zself": zself,
            "_own": own,
        })
    return (M, M2), tok, inv_tok, dinv, core_inputs


_BUILD_CACHE = {}
LAST_RESULT = None


def _build(Ms):
    """Build the SPMD Bass program (one NEFF shared by all 8 cores)."""
    M, M2 = Ms
    import os as _os
    no_cc = bool(int(_os.environ.get("KERNEL_NO_CC", "0")))
    key = (M, M2, no_cc)
    if key in _BUILD_CACHE:
        return _BUILD_CACHE[key]

    import concourse.bass as bass
    import concourse.mybir as mybir
    import concourse.tile as tile
    from concourse.bass import ts

    f16 = mybir.dt.float16
    f32 = mybir.dt.float32
    i16 = mybir.dt.int16
    AF = mybir.ActivationFunctionType
    OP = mybir.AluOpType

    NBH = WPC * M
    NBT = 2 * NBH
    PB = [0, 24, WPC]
    NBP = WPC * M2
    NBT2 = 2 * NBP

    from concourse import bacc
    nc = bacc.Bacc(num_devices=CORES, num_swdge_queues=4)

    ztab_d = nc.dram_tensor("ztab", [NTOK, D], f16, kind="ExternalInput")
    idx_d = nc.dram_tensor("idx16", [128, NBT * 8], i16, kind="ExternalInput")
    stiles_d = nc.dram_tensor("stilesT", [128, NBT * 128], f16, kind="ExternalInput")
    temb_d = nc.dram_tensor("temb", [128, SHARD], f16, kind="ExternalInput")
    diag_d = nc.dram_tensor("diagT", [128, SHARD], f16, kind="ExternalInput")
    zself_d = nc.dram_tensor("zself", [128, SHARD], f16, kind="ExternalInput")
    w1t_d = nc.dram_tensor("w1t", [D, D], f16, kind="ExternalInput")
    w2t_d = nc.dram_tensor("w2t", [D, D], f16, kind="ExternalInput")
    bias2_d = nc.dram_tensor("bias2", [1, D], f16, kind="ExternalInput")
    ones_d = nc.dram_tensor("ones", [1, D], f16, kind="ExternalInput")

    idx2_d = nc.dram_tensor("idx2", [128, NBT2 * 8], i16, kind="ExternalInput")
    stiles2_d = nc.dram_tensor("stiles2T", [128, NBT2 * 128], f16,
                               kind="ExternalInput")

    out_d = nc.dram_tensor("outshard", [SHARD, D], f32, kind="ExternalOutput")
    h1sp_d = [nc.dram_tensor(f"h1sp{q}", [(PB[q + 1] - PB[q]) * 128, D], f16)
              for q in range(2)]
    h1pf_d = [nc.dram_tensor(f"h1pf{q}", [CORES * (PB[q + 1] - PB[q]) * 128, D],
                             f16, addr_space="Shared") for q in range(2)]

    with tile.TileContext(nc) as tc:
        with (
            tc.tile_pool(name="const", bufs=1) as constp,
            tc.tile_pool(name="work", bufs=2) as workp,
            tc.tile_pool(name="psum", bufs=2, space="PSUM") as psump,
        ):
            def load_const(dram, shape, dtype, eng=None):
                t_ = constp.tile(shape, dtype, name=dram.name + "_t")
                (eng or nc.sync).dma_start(out=t_[:], in_=dram[:, :])
                return t_

            idx_t = load_const(idx_d, [128, NBT * 8], i16)
            idx2_t = load_const(idx2_d, [128, NBT2 * 8], i16)
            w1t_t = load_const(w1t_d, [D, D], f16)
            w2t_t = load_const(w2t_d, [D, D], f16)
            bias2_t = load_const(bias2_d, [1, D], f16)
            ones_t = load_const(ones_d, [1, D], f16)
            temb_t = load_const(temb_d, [128, SHARD], f16)
            diag_t = load_const(diag_d, [128, SHARD], f16)
            zself_t = load_const(zself_d, [128, SHARD], f16)

            aggbuf = constp.tile([128, SHARD], f32, name="aggbuf")
            h1stage = constp.tile([128, SHARD], f16, name="h1stage")

            def transform(L, w, aggT):
                # aggT: [f, t] f16; hp = aggT.T @ W.T -> [t, f']
                hp = psump.tile([128, D], f32, tag="hp", name="hp")
                wm = w1t_t if L == 0 else w2t_t
                nc.tensor.matmul(hp[:], lhsT=aggT[:], rhs=wm[:],
                                 start=True, stop=(L == 0))
                if L == 0:
                    # x = conv + temb ; h1 = elu(x) = relu(x)+exp(min(x,0))-1
                    x = workp.tile([128, D], f32, tag="x", name="x")
                    nc.vector.tensor_tensor(out=x[:], in0=hp[:],
                                            in1=temb_t[:, ts(w, D)], op=OP.add)
                    mn = workp.tile([128, D], f32, tag="mn", name="mn")
                    nc.vector.tensor_scalar(out=mn[:], in0=x[:],
                                            scalar1=0.0, scalar2=-60.0,
                                            op0=OP.min, op1=OP.max)
                    ex = workp.tile([128, D], f16, tag="ex", name="ex")
                    nc.scalar.activation(ex[:], mn[:], AF.Exp)
                    rl = workp.tile([128, D], f16, tag="rl", name="rl")
                    nc.scalar.activation(rl[:], x[:], AF.Relu)
                    e1 = workp.tile([128, D], f16, tag="e1", name="e1")
                    nc.vector.tensor_scalar(out=e1[:], in0=ex[:],
                                            scalar1=1.0, scalar2=None,
                                            op0=OP.subtract)
                    nc.vector.tensor_tensor(out=h1stage[:, ts(w, D)],
                                            in0=e1[:], in1=rl[:], op=OP.add)
                    p_ = 0 if w < PB[1] else 1
                    lw = w - PB[p_]
                    nc.sync.dma_start(
                        out=h1sp_d[p_][lw * 128:(lw + 1) * 128, :],
                        in_=h1stage[:, ts(w, D)])
                    if w == PB[p_ + 1] - 1:
                        if no_cc:
                            rows_ = (PB[p_ + 1] - PB[p_]) * 128
                            nc.sync.dma_start(out=h1pf_d[p_][0:rows_, :],
                                              in_=h1sp_d[p_][:, :])
                        else:
                            nc.gpsimd.collective_compute(
                                "AllGather",
                                mybir.AluOpType.bypass,
                                replica_groups=[list(range(CORES))],
                                ins=[h1sp_d[p_].ap().opt()],
                                outs=[h1pf_d[p_].ap().opt()],
                            )
                else:
                    nc.tensor.matmul(hp[:], lhsT=ones_t[:1, :],
                                     rhs=bias2_t[:1, :],
                                     start=False, stop=True)
                    ot = workp.tile([128, D], f32, tag="ot", name="ot")
                    nc.scalar.copy(out=ot[:], in_=hp[:])
                    nc.sync.dma_start(out=out_d[w * 128:(w + 1) * 128, :],
                                      in_=ot[:])

            gci = 0
            for L in range(2):
                selfsrc = zself_t if L == 0 else h1stage
                Mc = M if L == 0 else M2
                NBc = WPC * Mc
                idxc = idx_t if L == 0 else idx2_t
                stdram = stiles_d if L == 0 else stiles2_d
                for h in range(2):
                    if L == 0:
                        src_ap = ztab_d[:, :] if h == 0 else ztab_d[HALF:, :]
                    else:
                        src_ap = h1pf_d[h][:, :]
                    cur = None
                    stc = None
                    for ci_, c0 in enumerate(range(0, NBc, G)):
                        nbc = min(G, NBc - c0)
                        gb = h * NBc + c0
                        zg = workp.tile([128, nbc * D], f16, tag="zg",
                                        bufs=14, name="zg")
                        nc.gpsimd.dma_gather(
                            out_ap=zg[:].rearrange("p (b e) -> p b e", e=D),
                            in_ap=src_ap,
                            idxs_ap=idxc[:, gb * 8:(gb + nbc) * 8],
                            num_idxs=nbc * 128,
                            num_idxs_reg=nbc * 128,
                            elem_size=D,
                            queue_num=gci % 4,
                        )
                        gci += 1
                        if ci_ % 2 == 0:
                            nst = min(2 * G, NBc - c0)
                            stc = workp.tile([128, nst * D], f16, tag="stc",
                                             bufs=10, name="stc")
                            eng = nc.sync if (ci_ // 2) % 2 == 0 else nc.scalar
                            eng.dma_start(
                                out=stc[:],
                                in_=stdram[:, gb * 128:(gb + nst) * 128],
                            )
                            stc_base = c0
                        for jj in range(nbc):
                            blk = c0 + jj
                            sj = blk - stc_base
                            w = blk // Mc
                            jm = blk % Mc
                            if jm == 0:
                                cur = psump.tile([128, D], f32, tag="agg",
                                                 bufs=6, name="agg")
                                if h == 0:
                                    nc.tensor.matmul(
                                        cur[:], lhsT=selfsrc[:, ts(w, D)],
                                        rhs=diag_t[:, ts(w, D)],
                                        start=True, stop=False)
                            nc.tensor.matmul(
                                cur[:], lhsT=zg[:, ts(jj, D)],
                                rhs=stc[:, ts(sj, D)],
                                start=(jm == 0 and h == 1),
                                stop=(jm == Mc - 1),
                            )
                            if jm == Mc - 1:
                                if h == 0:
                                    nc.scalar.copy(out=aggbuf[:, ts(w, D)],
                                                   in_=cur[:])
                                else:
                                    aggT = workp.tile([128, D], f16,
                                                      tag="aggT", bufs=3,
                                                      name="aggT")
                                    nc.vector.tensor_tensor(
                                        out=aggT[:], in0=cur[:],
                                        in1=aggbuf[:, ts(w, D)], op=OP.add)
                                    transform(L, w, aggT)

    nc.finalize()
    _BUILD_CACHE[key] = nc
    return nc


def _temb_table(Wt1, bt1, Wt2, bt2, b1):
    """Exact time-MLP for all possible t values: [TMAX, D] float64 -> f32."""
    try:
        from scipy.special import erf
    except ImportError:
        import math
        erf = np.vectorize(math.erf)
    tv = np.arange(TMAX, dtype=np.float64)[:, None]          # [TMAX, 1]
    pre = tv @ np.asarray(Wt1, np.float64).T + np.asarray(bt1, np.float64)
    g = 0.5 * pre * (1.0 + erf(pre / np.sqrt(2.0)))
    emb = g @ np.asarray(Wt2, np.float64).T + np.asarray(bt2, np.float64)
    emb = emb + np.asarray(b1, np.float64)
    return emb.astype(np.float32)


def kernel(z, edge_index, t, Wt1, bt1, Wt2, bt2, W1, b1, W2, b2):
    z = np.asarray(z, dtype=np.float32)
    t = np.asarray(t)
    Ms, tok, inv_tok, dinv, core_inputs = _prep(z, edge_index, t)
    nc = _build(Ms)

    table = _temb_table(Wt1, bt1, Wt2, bt2, b1)

    ztab = np.zeros((NTOK, D), dtype=np.float16)
    ztab[tok] = z.astype(np.float16)

    shared = {
        "ztab": ztab,
        "w1t": np.ascontiguousarray(np.asarray(W1, np.float32).T).astype(np.float16),
        "w2t": np.ascontiguousarray(np.asarray(W2, np.float32).T).astype(np.float16),
        "bias2": np.asarray(b2, np.float32).astype(np.float16).reshape(1, D),
        "ones": np.ones((1, D), dtype=np.float16),
    }

    in_maps = []
    for k in range(CORES):
        ci = core_inputs[k]
        m = dict(shared)
        m["stilesT"] = ci["stilesT"]
        m["idx16"] = ci["idx16"]
        m["stiles2T"] = ci["stiles2T"]
        m["idx2"] = ci["idx2"]
        m["diagT"] = ci["diagT"]
        m["zself"] = ci["zself"]
        own = ci["_own"]
        te = np.zeros((SHARD, D), dtype=np.float16)
        ok = own >= 0
        te[ok] = table[np.asarray(t[own[ok]], np.int64)].astype(np.float16)
        m["temb"] = np.ascontiguousarray(
            te.reshape(WPC, 128, D).transpose(1, 0, 2).reshape(128, SHARD))
        in_maps.append(m)

    from concourse.bass_utils import run_bass_kernel_spmd
    res = run_bass_kernel_spmd(nc, in_maps, core_ids=list(range(CORES)))
    global LAST_RESULT
    LAST_RESULT = res
    out_tok = np.concatenate(
        [res.results[k]["outshard"] for k in range(CORES)], axis=0)
    return out_tok[tok].astype(np.float32)
